# revision 2
# baseline (speedup 1.0000x reference)
"""Depthwise 3D transposed conv (stride 2, k=4, SAME) on 8 trn2 NeuronCores.

x: (4, 32, 32, 32, 256) f32, filters: (4, 4, 4, 1, 256) f32
y: (4, 64, 64, 64, 256) f32

Sharding: 8 cores = (batch n in 4) x (d-halves in 2). Zero communication.

v2: double pair-fold. The d-dim AND h-dim tap pairs are both folded into
the matmul contraction:
  partitions p = j*64 + ih*32 + cc   (plane k+j, h-shift ih, channel cc of 32)
  cols      q = r*64 + rh*32 + c'    (d-parity r, h-parity rh, channel c')
  W[p, q] = delta(cc,c') * F[2(1-j)+r, 2(1-ih)+rh, kw, c']
Each col contracts 4 useful partitions -> 512 useful MACs/cycle (2x the
pair-fold baseline). The w-dim taps accumulate in PSUM (2 matmuls per
w-parity pw, windows shifted by dw).

Per (k, sigma-strip of 32ch): 2 pw x 2 taps x 3 a'-chunks = 12 matmuls of
[128x128] x [128, 11, 32]. Outputs evacuate PSUM->SBUF alternating
ScalarE/VectorE with f32->f16 cast; stores are f16 (halves out traffic).

Host pre-replicates the ih shift (2x input bytes; input is f16 so this is
39MB/core) and un-interleaves the f16 output.
"""
import sys

sys.path.insert(0, "/opt/trn_rl_repo")

from contextlib import ExitStack

import numpy as np

import concourse.bass as bass  # noqa: F401  (registers engine classes)
import concourse.tile as tile
from concourse import bacc, mybir
from concourse.bass_utils import run_bass_kernel_spmd

F32 = mybir.dt.float32
F16 = mybir.dt.float16

N_CORES = 8
# per-dim taps: parity -> [(delta, k), ...]
TAPS = {0: [(-1, 3), (0, 1)], 1: [(0, 2), (1, 0)]}
NK = 17  # plane-pair tiles per core: k holds local planes (k, k+1)

_PROG = None


def _build_program():
    nc = bacc.Bacc(
        "TRN2", target_bir_lowering=False, debug=False, num_devices=N_CORES
    )
    # xq[k, p=(j2,ih2,cc32), sig8, a'33, b'34] f16: partition-ready, h-shift
    # pre-replicated, halo pre-padded
    xq_d = nc.declare_dram_parameter("xq", [NK, 128, 8, 33, 34], F16, isOutput=False)
    # wq[sig, p, (pw,t), col] f16
    wq_d = nc.declare_dram_parameter("wq", [8, 128, 4, 128], F16, isOutput=False)
    # yq[k, p=(r,rh,c'), sig, pw, a', b] f16
    yq_d = nc.declare_dram_parameter("yq", [NK, 128, 8, 2, 33, 32], F16, isOutput=True)

    with ExitStack() as ctx:
        tc = ctx.enter_context(tile.TileContext(nc))
        wpool = ctx.enter_context(tc.tile_pool(name="wpool", bufs=1))
        xpool = ctx.enter_context(tc.tile_pool(name="xpool", bufs=3))
        opool = ctx.enter_context(tc.tile_pool(name="opool", bufs=3))
        ppool = ctx.enter_context(tc.tile_pool(name="ppool", bufs=8, space="PSUM"))

        wd = wpool.tile([128, 32, 128], F16)
        wd_loaded = set()

        def load_wchunk(sig):
            if sig not in wd_loaded:
                nc.sync.dma_start(
                    out=wd[:, sig * 4 : (sig + 1) * 4, :], in_=wq_d[sig]
                )
                wd_loaded.add(sig)

        for k in range(NK):
            xt = xpool.tile([128, 8, 33, 34], F16, tag="xp")
            if k == 0:
                # split the first load so matmuls start after ~0.6MB, not 2.3MB
                for g in range(4):
                    gs = slice(g * 2, (g + 1) * 2)
                    nc.sync.dma_start(out=xt[:, gs], in_=xq_d[k, :, gs])
            else:
                nc.sync.dma_start(out=xt, in_=xq_d[k])
            ot = opool.tile([128, 8, 2, 33, 32], F16, tag="out")
            for sig in range(8):
                load_wchunk(sig)
                pss = [
                    ppool.tile([128, 11, 32], F32, tag="ps", name="ps")
                    for _ in range(6)
                ]
                for pw in range(2):
                    for t in range(2):
                        dw = TAPS[pw][t][0]
                        wap = wd[:, sig * 4 + pw * 2 + t, :]
                        for c in range(3):
                            nc.tensor.matmul(
                                pss[pw * 3 + c],
                                wap,
                                xt[:, sig, c * 11 : c * 11 + 11, 1 + dw : 33 + dw],
                                start=(t == 0),
                                stop=(t == 1),
                            )
                for i, (pw, c) in enumerate(
                    [(p, c) for p in range(2) for c in range(3)]
                ):
                    dst = ot[:, sig, pw, c * 11 : c * 11 + 11, :]
                    if i % 2 == 0:
                        nc.scalar.copy(dst, pss[pw * 3 + c])
                    else:
                        nc.vector.tensor_copy(dst, pss[pw * 3 + c])
            # SWDGE stores keep the Sync FIFO free for loads. Boundary k
            # stores only the valid r half (r=0 cols are parts 0:64).
            # Stores go out in two sigma halves so the first half ships
            # while the second half is still being evacuated.
            for g in range(2):
                gs = slice(g * 4, (g + 1) * 4)
                if k == 0:
                    nc.gpsimd.dma_start(
                        out=yq_d[k, 64:128, gs], in_=ot[64:128, gs]
                    )
                elif k == NK - 1:
                    nc.gpsimd.dma_start(out=yq_d[k, 0:64, gs], in_=ot[0:64, gs])
                else:
                    nc.gpsimd.dma_start(out=yq_d[k, :, gs], in_=ot[:, gs])
    nc.compile()
    return nc


def _get_program():
    global _PROG
    if _PROG is None:
        _PROG = _build_program()
    return _PROG


def _make_in_maps(x, filters):
    x = np.asarray(x, dtype=np.float32)
    filters = np.asarray(filters, dtype=np.float32)
    ftap = filters[:, :, :, 0, :]  # (kd, kh, kw, c)

    # weight table: wq[sig, p=(j,ih,cc), (pw,t), col=(r,rh,c')] =
    #   delta(cc,c') * F[2(1-j)+r, 2(1-ih)+rh, kw(pw,t), sig*32+c']
    wq = np.zeros((8, 128, 4, 128), np.float16)
    idx = np.arange(32)
    for sig in range(8):
        for pw in range(2):
            for t in range(2):
                kw = TAPS[pw][t][1]
                for j in range(2):
                    for ih in range(2):
                        for r in range(2):
                            for rh in range(2):
                                kd = 2 * (1 - j) + r
                                kh = 2 * (1 - ih) + rh
                                wq[
                                    sig,
                                    j * 64 + ih * 32 + idx,
                                    pw * 2 + t,
                                    r * 64 + rh * 32 + idx,
                                ] = ftap[kd, kh, kw, sig * 32 + idx]

    in_maps = []
    for core in range(N_CORES):
        n, hh = core // 2, core % 2
        lo = 16 * hh - 1
        # xpad[ch, i, hp, wp] = x[n, lo+i, hp-1, wp-1, ch], zero-padded
        xpad = np.zeros((256, 18, 34, 34), np.float32)
        s0, s1 = max(lo, 0), min(lo + 18, 32)
        xpad[:, s0 - lo : s1 - lo, 1:33, 1:33] = x[n, s0:s1].transpose(3, 0, 1, 2)
        xpv = xpad.reshape(8, 32, 18, 34, 34)  # (sig, cc, i, hp, wp)
        xq = np.empty((NK, 128, 8, 33, 34), np.float16)
        for j in range(2):
            for ih in range(2):
                p0 = j * 64 + ih * 32
                # dst (k, cc, sig, a', b') <- src (sig, cc, k+j, a'+ih, b')
                xq[:, p0 : p0 + 32] = xpv[:, :, j : j + NK, ih : ih + 33].transpose(
                    2, 1, 0, 3, 4
                )
        in_maps.append({"xq": xq, "wq": wq})
    return in_maps


def kernel(x, filters):
    nc = _get_program()
    in_maps = _make_in_maps(x, filters)
    res = run_bass_kernel_spmd(nc, in_maps, list(range(N_CORES)))
    y = np.empty((4, 64, 64, 64, 256), np.float32)
    for core in range(N_CORES):
        n, hh = core // 2, core % 2
        yq = res.results[core]["yq"]  # (k, (r,rh,c'), sig, pw, a', b) f16
        yc = yq.reshape(NK, 2, 2, 32, 8, 2, 33, 32)
        # (k, r, a', rh, b, pw, sig, c'); d=2k+r-1, ho=2a'+rh-1, wo=2b+pw
        yc = yc.transpose(0, 1, 6, 2, 7, 5, 4, 3).reshape(34, 66, 64, 256)
        y[n, 32 * hh : 32 * hh + 32] = yc[1:33, 1:65]
    return y
